# revision 1
# baseline (speedup 1.0000x reference)
"""Trainium2 Bass kernel for nn_Actor_attf (gnn_message_passing).

Data-parallel over batch across 8 NeuronCores. Per core: 32768 rows
(agent, batch) x 96 features, fully flattened (weights shared across agents).

Layout: features-on-partitions, rows streaming in the free dimension.
All matmuls bf16 (1 cyc/row); input DMA'd as bf16 (halves HBM traffic).
Encoders via block-diagonal packed weights; attention via mask matmuls.
LayerNorm uses centered two-pass stats (mean first, then E[(x-mu)^2]) so
bf16 rounding does not cancel; rstd via quake rsqrt + 1 Newton step on DVE.
Single ACT table set (exp_and_others: relu/exp/tanh/square/parametric_relu).

PE tile-position rules respected: matmul SBUF operands at base partition
{0,32,64} with lhsT.base == rhs.base; psum outputs quadrant-aligned and
zero-padded (via zero lhsT columns) so no stale PSUM is ever read.
"""
import numpy as np
import ml_dtypes
from contextlib import ExitStack

import concourse.bass as bass
import concourse.tile as tile
from concourse import mybir
from concourse.bass_utils import run_bass_kernel_spmd

F32 = mybir.dt.float32
BF16 = mybir.dt.bfloat16
I32 = mybir.dt.int32
AF = mybir.ActivationFunctionType
OP = mybir.AluOpType
BF = ml_dtypes.bfloat16

A, B, OBS, NU, NT = 16, 16384, 96, 32, 16
M = 8              # cores
Bs = B // M        # 2048 batch per core
R = A * Bs         # 32768 rows per core
NSUB = 1024        # rows per subtile (two psum banks)
NMH = 512          # matmul half width (one psum bank)
NMAC = 2048        # rows per macro tile
NMACROS = R // NMAC
EPS = 1e-5
QMAGIC = 0x5F3759DF


def bd(w, k):
    """block-diag k copies of w."""
    ki, ko = w.shape
    out = np.zeros((ki * k, ko * k), np.float32)
    for g in range(k):
        out[ki * g:ki * (g + 1), ko * g:ko * (g + 1)] = w
    return out


class ColPack:
    """Constant matrices packed as column blocks of one [128, W] array.

    Content placed at rows [row0:row0+k]; kernel slices [sbase:sbase+ssize]."""

    def __init__(self):
        self.cols = []
        self.off = 0
        self.idx = {}

    def add(self, name, arr, row0=0, sbase=0, ssize=None):
        arr = np.asarray(arr, np.float32)
        if arr.ndim == 1:
            arr = arr[:, None]
        k, m = arr.shape
        if ssize is None:
            ssize = row0 + k - sbase
        a = np.zeros((128, m), np.float32)
        a[row0:row0 + k] = arr
        self.idx[name] = (self.off, sbase, ssize, m)
        self.cols.append(a)
        self.off += m

    def pack(self):
        return np.concatenate(self.cols, axis=1)


def build_consts(w):
    """Returns (cpb, cpf): bf16 matmul lhsTs and f32 bias/misc columns."""
    cpb = ColPack()
    cpf = ColPack()
    oa_w1, oa_w2 = w["oa_w1"], w["oa_w2"]
    g_w1, g_w2 = w["g_w1"], w["g_w2"]
    en_w1, en_w2 = w["en_w1"], w["en_w2"]
    seps = 4.0 * np.sqrt(EPS)

    # ---- L1 lhsTs: window = 32 input-feature partitions ----
    cpb.add("w0a", bd(oa_w1, 4), row0=0, sbase=0, ssize=32)     # oa g0-3
    cpb.add("w0b", bd(oa_w1, 4), row0=16, sbase=0, ssize=32)    # oa g4-7
    cpb.add("w1c", bd(oa_w1, 4), row0=32, sbase=32, ssize=32)   # oa g8-11
    cpb.add("w1d", bd(oa_w1, 3), row0=48, sbase=32, ssize=32)   # oa g12-14
    cpb.add("w1s", en_w1, row0=60, sbase=32, ssize=32)          # self
    cpb.add("w2a", bd(g_w1, 4), row0=64, sbase=64, ssize=32)    # goal g0-3
    cpb.add("w2b", bd(g_w1, 4), row0=72, sbase=64, ssize=32)    # g4-7
    cpb.add("w2c", bd(g_w1, 4), row0=80, sbase=64, ssize=32)    # g8-11
    cpb.add("w2d", bd(g_w1, 4), row0=88, sbase=64, ssize=32)    # g12-15
    # ---- L2 lhsTs ----
    cpb.add("lw_oa2", bd(oa_w2, 4))            # [128,64]
    cpb.add("lw_oa2c", bd(oa_w2, 3))           # [96,48]
    cpb.add("lw_en2", en_w2)                   # [32,16]
    cpb.add("lw_g2", bd(g_w2, 4))              # [128,64]
    # ---- attention ----
    r16 = np.zeros((16, 128), np.float32)
    for j in range(8):
        for u in range(16):
            r16[u, 16 * j + u] = 1.0
    cpb.add("r16", r16)
    m8 = np.zeros((128, 32), np.float32)      # scores mask (8 real cols)
    for j in range(8):
        m8[16 * j:16 * j + 16, j] = 1.0
    cpb.add("m8w", m8)
    m8b = np.zeros((112, 32), np.float32)     # oaB: 7 groups at cols 8:15
    for j in range(7):
        m8b[16 * j:16 * j + 16, 8 + j] = 1.0
    cpb.add("m8bw", m8b)
    # e-replicate lhsTs: e lives at psc rows {0:8, 32:40, 64:72, 72:79}
    for nm, base, nj, ncol in [("e_ga", 0, 8, 128), ("e_gb", 32, 8, 128),
                               ("e_oaa", 64, 8, 128), ("e_oab", 72, 7, 112)]:
        e = np.zeros((96, ncol), np.float32)
        for j in range(nj):
            for u in range(16):
                e[base + j, 16 * j + u] = 1.0
        cpb.add(nm, e)
    u16 = np.zeros((128, 32), np.float32)     # centered wsum mask
    for j in range(8):
        for u in range(16):
            for u2 in range(16):
                u16[16 * j + u, u2] = (1.0 if u == u2 else 0.0) - 1.0 / 16.0
    cpb.add("u16w", u16)
    cpb.add("u16bw", u16[:112, :].copy())
    # ---- LN stats (centered two-pass) ----
    stmu = np.zeros((64, 32), np.float32)
    stmu[0:16, 0] = 1.0 / 16.0     # mu_goal
    stmu[32:48, 1] = 1.0 / 16.0    # mu_oa
    cpb.add("stmu", stmu)
    stde = np.zeros((96, 32), np.float32)
    stde[0:8, 0] = seps
    stde[32:40, 0] = seps          # goal denom: e rows 0:8 + 32:40
    stde[64:79, 1] = seps          # oa denom: e rows 64:79
    cpb.add("stdew", stde)
    sts2 = np.zeros((64, 32), np.float32)
    sts2[0:16, 0] = 1.0
    sts2[32:48, 1] = 1.0
    cpb.add("sts2w", sts2)         # sum of (x-mu)^2 -> 16*var
    id2 = np.zeros((2, 32), np.float32)
    id2[0, 0] = 1.0
    id2[1, 1] = 1.0
    cpb.add("id2", id2)            # accumulate De^2 into R
    bcmu = np.zeros((2, 64), np.float32)
    bcmu[0, 0:16] = 1.0
    bcmu[1, 32:48] = 1.0
    cpb.add("bcmu", bcmu)
    bcrg = np.zeros((2, 64), np.float32)
    bcrg[0, 0:16] = 4.0 * w["g_ln_g"]
    bcrg[1, 32:48] = 4.0 * w["oa_ln_g"]
    cpb.add("bcrg", bcrg)          # rstd = 4/sqrt(R16); 4 folded here
    # ---- actor ----
    cpb.add("aw1s", w["a_w1"][0:16])           # [16,32] self part
    aw1a = np.zeros((64, 32), np.float32)
    aw1a[0:16] = w["a_w1"][16:32]              # food
    aw1a[32:48] = w["a_w1"][32:48]             # other
    cpb.add("aw1a", aw1a)
    cpb.add("aw2", w["a_w2"])
    cpb.add("aw3", w["a_w3"])
    # ---- f32 biases + misc ----
    cpf.add("b1_oa", np.tile(w["oa_b1"], 4))
    cpf.add("b1_oac", np.tile(w["oa_b1"], 3))            # [96]
    cpf.add("b1_self", w["en_b1"])                       # [32]
    cpf.add("b1_g", np.tile(w["g_b1"], 4))
    cpf.add("b2_oa", np.tile(w["oa_b2"], 8))
    cpf.add("b2_oab", np.tile(w["oa_b2"], 7))            # [112]
    cpf.add("b2_self", w["en_b2"])                       # [16]
    cpf.add("b2_g", np.tile(w["g_b2"], 8))
    beta64 = np.zeros((64,), np.float32)
    beta64[0:16] = w["g_ln_b"]
    beta64[32:48] = w["oa_ln_b"]
    cpf.add("beta64", beta64)
    cpf.add("ab1", w["a_b1"])
    cpf.add("ab2", w["a_b2"])
    cpf.add("ab3", w["a_b3"])
    cpf.add("qshift", np.full((2, 1), 1, np.int32).view(np.float32))
    cpf.add("qxor", np.full((2, 1), -1, np.int32).view(np.float32))
    cpf.add("qmag", np.full((2, 1), float(QMAGIC + 1), np.float32))
    return cpb, cpf


def perm96():
    """row r of device X = feature perm[r] of reference obs vector."""
    p = []
    for g in range(15):  # oa groups: (pos2, vel2)
        p += [4 + 2 * g, 5 + 2 * g, 34 + 2 * g, 35 + 2 * g]
    p += [0, 1, 2, 3]        # self at rows 60:64
    p += list(range(64, 96))  # goal pairs at 64:96
    return np.array(p)


# ---------------------------------------------------------------- graph
def _emit(nc, tc, ctx, x, cwb, cwf, out, idxb, idxf, nb, nf):
    const = ctx.enter_context(tc.tile_pool(name="const", bufs=1))
    pin = ctx.enter_context(tc.tile_pool(name="pin", bufs=3))
    ph1p = ctx.enter_context(tc.tile_pool(name="ph1", bufs=9))
    penc = ctx.enter_context(tc.tile_pool(name="penc", bufs=6))
    pmul = ctx.enter_context(tc.tile_pool(name="pmul", bufs=6))
    ppn = ctx.enter_context(tc.tile_pool(name="ppn", bufs=6))
    pmid = ctx.enter_context(tc.tile_pool(name="pmid", bufs=2))
    pnar = ctx.enter_context(tc.tile_pool(name="pnar", bufs=2))
    pout = ctx.enter_context(tc.tile_pool(name="pout", bufs=2))
    pps = ctx.enter_context(tc.tile_pool(name="pps", bufs=4, space="PSUM"))

    cwb_s = const.tile([128, nb], BF16)
    nc.gpsimd.dma_start(out=cwb_s, in_=cwb[:, :])
    cwf_s = const.tile([128, nf], F32)
    nc.gpsimd.dma_start(out=cwf_s, in_=cwf[:, :])

    def cc(name):
        off, sbase, ssize, m_ = idxb[name]
        return cwb_s[sbase:sbase + ssize, off:off + m_]

    def ccb(name, n):  # f32 bias column, rows 0:n
        off, sbase, ssize, m_ = idxf[name]
        return cwf_s[0:n, off:off + 1]

    def mm(o, lhsT, rhs, start=True, stop=True):
        for h in range(NSUB // NMH):
            nc.tensor.matmul(o[:, h * NMH:(h + 1) * NMH], lhsT,
                             rhs[:, h * NMH:(h + 1) * NMH],
                             start=start, stop=stop)

    def drain_relu(dst, src, bias, n, use_act):
        if use_act:
            nc.scalar.activation(dst, src, AF.Relu, bias=ccb(bias, n))
        else:
            nc.vector.tensor_scalar(out=dst, in0=src, scalar1=ccb(bias, n),
                                    scalar2=0.0, op0=OP.add, op1=OP.max)

    for imac in range(NMACROS):
        xin = pin.tile([96, NMAC], BF16, tag="xin")
        nc.sync.dma_start(out=xin, in_=x[:, imac * NMAC:(imac + 1) * NMAC])
        outw = pout.tile([2, NMAC], F32, tag="outw")

        for isub in range(NMAC // NSUB):
            s0 = isub * NSUB
            xs = xin[:, s0:s0 + NSUB]
            x0, x1, x2 = xs[0:32, :], xs[32:64, :], xs[64:96, :]

            # ---------------- L1: 9 matmuls, 9 drains ----------------
            h1 = []
            specs = [("w0a", x0, "b1_oa", 128), ("w0b", x0, "b1_oa", 128),
                     ("w1c", x1, "b1_oa", 128), ("w1d", x1, "b1_oac", 96),
                     ("w2a", x2, "b1_g", 128), ("w2b", x2, "b1_g", 128),
                     ("w2c", x2, "b1_g", 128), ("w2d", x2, "b1_g", 128)]
            for i, (lw, xw, bias, npart) in enumerate(specs):
                ps = pps.tile([128, NSUB], F32, tag="ps")
                mm(ps[0:npart, :], cc(lw), xw)
                hs = ph1p.tile([128, NSUB], BF16, tag="h1")
                drain_relu(hs[0:npart, :], ps[0:npart, :], bias, npart,
                           use_act=(i % 2 == 0))
                h1.append(hs)
            psq2 = pps.tile([32, NSUB], F32, tag="ps")
            mm(psq2, cc("w1s"), x1)
            hq = pmid.tile([32, NSUB], BF16, tag="hq")
            drain_relu(hq, psq2, "b1_self", 32, use_act=False)

            # ---------------- L2: 9 matmuls, 5 drains ----------------
            psA = pps.tile([128, NSUB], F32, tag="ps")
            mm(psA[0:64, :], cc("lw_oa2"), h1[0])
            mm(psA[64:128, :], cc("lw_oa2"), h1[1])
            encA = penc.tile([128, NSUB], BF16, tag="enc")
            nc.scalar.activation(encA, psA, AF.Relu, bias=ccb("b2_oa", 128))
            psB = pps.tile([128, NSUB], F32, tag="ps")
            mm(psB[0:64, :], cc("lw_oa2"), h1[2])
            mm(psB[64:112, :], cc("lw_oa2c"), h1[3][0:96, :])
            encB = penc.tile([112, NSUB], BF16, tag="encb")
            nc.vector.tensor_scalar(out=encB, in0=psB[0:112, :],
                                    scalar1=ccb("b2_oab", 112), scalar2=0.0,
                                    op0=OP.add, op1=OP.max)
            psq = pps.tile([16, NSUB], F32, tag="ps")
            mm(psq, cc("lw_en2"), hq)
            q_s = pmid.tile([16, NSUB], BF16, tag="qs")
            nc.scalar.activation(q_s, psq, AF.Relu, bias=ccb("b2_self", 16))
            psGA = pps.tile([128, NSUB], F32, tag="ps")
            mm(psGA[0:64, :], cc("lw_g2"), h1[4])
            mm(psGA[64:128, :], cc("lw_g2"), h1[5])
            encGA = penc.tile([128, NSUB], BF16, tag="enc")
            nc.scalar.activation(encGA, psGA, AF.Relu, bias=ccb("b2_g", 128))
            psGB = pps.tile([128, NSUB], F32, tag="ps")
            mm(psGB[0:64, :], cc("lw_g2"), h1[6])
            mm(psGB[64:128, :], cc("lw_g2"), h1[7])
            encGB = penc.tile([128, NSUB], BF16, tag="enc")
            nc.vector.tensor_scalar(out=encGB, in0=psGB,
                                    scalar1=ccb("b2_g", 128), scalar2=0.0,
                                    op0=OP.add, op1=OP.max)

            # -------------- attention scores -------------------------
            psqr = pps.tile([128, NSUB], F32, tag="ps")
            mm(psqr, cc("r16"), q_s)
            qrep = pmid.tile([128, NSUB], BF16, tag="qrep")
            nc.scalar.activation(qrep, psqr, AF.Copy, scale=0.25)
            psc = pps.tile([96, NSUB], F32, tag="ps")
            pga = pmul.tile([128, NSUB], BF16, tag="pm")
            nc.vector.tensor_mul(out=pga, in0=encGA, in1=qrep)
            mm(psc[0:32, :], cc("m8w"), pga)
            pgb = pmul.tile([128, NSUB], BF16, tag="pm")
            nc.vector.tensor_mul(out=pgb, in0=encGB, in1=qrep)
            mm(psc[32:64, :], cc("m8w"), pgb)
            poa = pmul.tile([128, NSUB], BF16, tag="pm")
            nc.vector.tensor_mul(out=poa, in0=encA, in1=qrep)
            mm(psc[64:96, :], cc("m8w"), poa, start=True, stop=False)
            pob = pmul.tile([112, NSUB], BF16, tag="pm")
            nc.vector.tensor_mul(out=pob, in0=encB, in1=qrep[0:112, :])
            mm(psc[64:96, :], cc("m8bw"), pob, start=False, stop=True)
            e_s = pmid.tile([96, NSUB], BF16, tag="es")
            nc.scalar.activation(e_s, psc, AF.Exp)

            # -------------- weighted sums ----------------------------
            att = pps.tile([64, NSUB], F32, tag="ps")
            wspec = [("e_ga", "u16w", encGA, 128, 0, True),
                     ("e_gb", "u16w", encGB, 128, 0, False),
                     ("e_oaa", "u16w", encA, 128, 32, True),
                     ("e_oab", "u16bw", encB, 112, 32, False)]
            for elh, ulh, enc_t, np_, ro, st in wspec:
                per = pps.tile([128, NSUB], F32, tag="ps")
                mm(per[0:np_, :], cc(elh), e_s)
                pp = ppn.tile([128, NSUB], BF16, tag="pp")
                nc.vector.tensor_mul(out=pp[0:np_, :], in0=enc_t,
                                     in1=per[0:np_, :])
                mm(att[ro:ro + 32, :], cc(ulh), pp[0:np_, :],
                   start=st, stop=not st)

            # ---- LN: att is already mean-centered (mask carries -1/16) ----
            d = pmid.tile([64, NSUB], F32, tag="d")
            nc.vector.tensor_scalar_add(out=d, in0=att, scalar1=0.0)
            sqd = pmid.tile([64, NSUB], BF16, tag="sqd")
            nc.scalar.activation(sqd, att, AF.Square)
            psde = pps.tile([32, NSUB], F32, tag="ps")
            mm(psde, cc("stdew"), e_s)
            deb = pnar.tile([2, NSUB], BF16, tag="deb")
            nc.scalar.activation(deb, psde[0:2, :], AF.Copy)
            de2 = pnar.tile([2, NSUB], BF16, tag="de2")
            nc.vector.tensor_mul(out=de2, in0=deb, in1=deb)
            prv = pps.tile([32, NSUB], F32, tag="ps")
            mm(prv, cc("sts2w"), sqd, start=True, stop=False)
            mm(prv, cc("id2"), de2, start=False, stop=True)
            # quake rsqrt + 1 newton step (f32, narrow)
            yi = pnar.tile([2, NSUB], I32, tag="yi")
            nc.vector.tensor_scalar(out=yi, in0=prv[0:2, :].bitcast(I32),
                                    scalar1=ccb("qshift", 2).bitcast(I32),
                                    scalar2=None, op0=OP.logical_shift_right)
            nc.vector.tensor_scalar(out=yi, in0=yi,
                                    scalar1=ccb("qxor", 2).bitcast(I32),
                                    scalar2=None, op0=OP.bitwise_xor)
            nc.vector.tensor_scalar(out=yi, in0=yi, scalar1=ccb("qmag", 2),
                                    scalar2=None, op0=OP.add)
            y0 = yi.bitcast(F32)
            t1 = pnar.tile([2, NSUB], F32, tag="t1")
            nc.vector.tensor_mul(out=t1, in0=y0, in1=y0)
            nc.vector.tensor_mul(out=t1, in0=t1, in1=prv[0:2, :])
            nc.vector.tensor_scalar(out=t1, in0=t1, scalar1=-0.5, scalar2=1.5,
                                    op0=OP.mult, op1=OP.add)
            y1 = pnar.tile([2, NSUB], BF16, tag="y1")
            nc.vector.tensor_mul(out=y1, in0=y0, in1=t1)
            pbr = pps.tile([64, NSUB], F32, tag="ps")
            mm(pbr, cc("bcrg"), y1)
            nc.vector.tensor_mul(out=d, in0=d, in1=pbr)
            mn = pmid.tile([64, NSUB], BF16, tag="mn")
            nc.scalar.activation(mn, d, AF.Relu, bias=ccb("beta64", 64))

            # -------------- actor ------------------------------------
            ph = pps.tile([32, NSUB], F32, tag="ps")
            mm(ph, cc("aw1s"), q_s, start=True, stop=False)
            mm(ph, cc("aw1a"), mn, start=False, stop=True)
            a1 = pmid.tile([32, NSUB], BF16, tag="a1")
            nc.scalar.activation(a1, ph, AF.Prelu, bias=ccb("ab1", 32),
                                 alpha=0.01)
            ph2 = pps.tile([32, NSUB], F32, tag="ps")
            mm(ph2, cc("aw2"), a1)
            a2 = pmid.tile([32, NSUB], BF16, tag="a2")
            nc.scalar.activation(a2, ph2, AF.Prelu, bias=ccb("ab2", 32),
                                 alpha=0.01)
            po = pps.tile([2, NSUB], F32, tag="ps")
            mm(po, cc("aw3"), a2)
            nc.scalar.activation(outw[:, s0:s0 + NSUB], po, AF.Tanh,
                                 bias=ccb("ab3", 2))

        nc.sync.dma_start(out=out[:, imac * NMAC:(imac + 1) * NMAC], in_=outw)




def _split_excess_waits(nc):
    """Walrus in this build can encode only 1 sync wait on Activation/PE
    instruction descriptors. Move extra waits onto a NoOp just before."""
    from concourse import mybir as _mb
    nsplit = 0
    for fn in nc.m.functions:
        for bb in fn.blocks:
            insts = list(bb.instructions)
            out = []
            for ins in insts:
                si = ins.sync_info
                ow = list(si.on_wait) if (si is not None and si.on_wait) else []
                if len(ow) > 1:
                    for w0 in ow[:-1]:
                        nsplit += 1
                        nop = _mb.InstNoOp(
                            name=f"I-wsplit-{nsplit}",
                            engine=ins.engine,
                            ins=[], outs=[],
                            sync_info=_mb.SyncInfo(on_wait=[w0], on_update=[]),
                        )
                        nc.register_instruction(nop, overwrite=True)
                        out.append(nop)
                    ins.sync_info = _mb.SyncInfo(on_wait=[ow[-1]],
                                                 on_update=list(si.on_update or []))
                out.append(ins)
            if len(out) != len(insts):
                bb.instructions = out
    return nsplit


def build(nb, nf, idxb, idxf):
    nc = bass.Bass()
    x = nc.dram_tensor("x", [96, R], BF16, kind="ExternalInput")
    cwb = nc.dram_tensor("cwb", [128, nb], BF16, kind="ExternalInput")
    cwf = nc.dram_tensor("cwf", [128, nf], F32, kind="ExternalInput")
    out = nc.dram_tensor("out", [2, R], F32, kind="ExternalOutput")
    with tile.TileContext(nc) as tc, ExitStack() as ctx:
        _emit(nc, tc, ctx, x, cwb, cwf, out, idxb, idxf, nb, nf)
    n = _split_excess_waits(nc)
    print(f"split {n} multi-wait instructions")
    return nc


# ---------------------------------------------------------------- host API
def _prepare(inputs):
    w = {k: np.asarray(v, np.float32) for k, v in inputs.items()}
    cpb, cpf = build_consts(w)
    cwb = cpb.pack().astype(BF)
    cwf = cpf.pack()
    p = perm96()
    s = w["s_input"]  # [A, B, 96]
    in_maps = []
    for m in range(M):
        xs = s[:, m * Bs:(m + 1) * Bs, :].reshape(A * Bs, OBS)
        xt = np.ascontiguousarray(xs[:, p].T.astype(BF))  # [96, R] bf16
        in_maps.append({"x": xt, "cwb": cwb, "cwf": cwf})
    return (cwb.shape[1], cwf.shape[1], cpb.idx, cpf.idx), in_maps


def _assemble(results):
    out = np.empty((A, B, 2), np.float32)
    for m in range(M):
        o = np.asarray(results[m]["out"])  # [2, R]
        out[:, m * Bs:(m + 1) * Bs, :] = o.T.reshape(A, Bs, 2)
    return out


_nc_cache = {}


def _get_nc(key):
    if key[:2] not in _nc_cache:
        _nc_cache[key[:2]] = build(*key)
    return _nc_cache[key[:2]]


def kernel(**inputs):
    key, in_maps = _prepare(inputs)
    nc = _get_nc(key)
    res = run_bass_kernel_spmd(nc, in_maps, core_ids=list(range(M)))
    return _assemble(res.results)


def run_traced(inputs, **kw):
    key, in_maps = _prepare(inputs)
    nc = _get_nc(key)
    res = run_bass_kernel_spmd(nc, in_maps, core_ids=list(range(M)), **kw)
    return _assemble(res.results), res

